# revision 11
# baseline (speedup 1.0000x reference)
"""Trainium2 Bass kernel for nn_AttPool (4-layer GNN + additive-attention pooling).

Strategy (data-parallel over graphs, 32 graphs per NeuronCore):
  * Host re-lays-out the edge list as per-graph dense normalized adjacency
    Ahat^T = ((A + I) / deg)^T, shipped in bf16 (layer 0) and fp8-e4m3
    (layers 1-3).
  * Per graph g, per layer l:
      - aggT = sum_c h_c^T @ AhatT_c.  Layer 0 runs in bf16 (4 matmuls,
        h0 stays bf16 — quantizing the raw features dominates fp8 error);
        layers 1-3 run as 2 fp8 DoubleRow matmuls (K=256 per pass, both
        operands fp8) at ~2x column throughput.
      - linT = convW_l^T @ aggT (feat-major, single matmul, single weight
        load), then one tanh ACT emits hT_l directly in fp8.
      - node-major x_l (fp8) comes from 4 PE transposes of hT_l (post-tanh,
        so no second activation), feeding the next layer's spmm and pooling.
  * Attention: uT_m accumulated via fp8 DoubleRow matmuls (attW pre-packed
    x64 in fp8, descaled inside the tanh ACT's scale); t in two fused
    [128,1024] ACTs; scores via 4 column-packed M=1 matmuls + DVE row adds;
    softmax per graph; pooling via column-packed matmuls + DVE row adds.
  * Tiny epilogue: transpose pooled rows, output head matmul, ReLU.
  Measured end-to-end rel-err vs fp32 reference: ~1e-2 (gate 2e-2).
"""

import numpy as np
import ml_dtypes

B, N, F = 256, 512, 128
NL = 4
D = 512
OUT = 128
NCORES = 8
GPC = B // NCORES  # graphs per core

BF16 = ml_dtypes.bfloat16
FP8 = ml_dtypes.float8_e4m3
ATT_SCALE = 64.0

_NC_CACHE = {}


def _build_nc(has_conv_b, has_att_b, has_out_b):
    key = (has_conv_b, has_att_b, has_out_b)
    if key in _NC_CACHE:
        return _NC_CACHE[key]

    import concourse.bacc as bacc
    import concourse.tile as tile
    import concourse.mybir as mybir
    from concourse.masks import make_identity

    f32 = mybir.dt.float32
    bf16 = mybir.dt.bfloat16
    fp8 = mybir.dt.float8e4
    DR = mybir.MatmulPerfMode.DoubleRow

    nc = bacc.Bacc(None, target_bir_lowering=False)

    atb_d = nc.dram_tensor("atb", [GPC, 128, 4, D], bf16, kind="ExternalInput")
    at8_d = nc.dram_tensor("at8", [GPC, 128, 4, D], fp8, kind="ExternalInput")
    h0_d = nc.dram_tensor("h0", [GPC, 128, 4, F], bf16, kind="ExternalInput")
    convw_d = nc.dram_tensor("convw", [128, NL, F], bf16, kind="ExternalInput")
    attw_d = nc.dram_tensor("attw", [128, 16, F], fp8, kind="ExternalInput")
    attv_d = nc.dram_tensor("attv", [128, 4], bf16, kind="ExternalInput")
    outw_d = nc.dram_tensor("outw", [128, 4 * OUT], bf16, kind="ExternalInput")
    out_d = nc.dram_tensor("out", [GPC, OUT], f32, kind="ExternalOutput")
    convb_d = recip_d = attb_d = outb_d = None
    if has_conv_b:
        convb_d = nc.dram_tensor("convb", [1, NL * F], f32, kind="ExternalInput")
        recip_d = nc.dram_tensor("recipdeg", [GPC, D], f32, kind="ExternalInput")
    if has_att_b:
        attb_d = nc.dram_tensor("attb", [128, 4], f32, kind="ExternalInput")
    if has_out_b:
        outb_d = nc.dram_tensor("outb", [1, OUT], f32, kind="ExternalInput")

    with tile.TileContext(nc) as tc:
        with (
            tc.tile_pool(name="singles", bufs=1) as singles,
        ):
            convw_sb = singles.tile([128, NL, F], bf16)
            attw_sb = singles.tile([128, 16, F], fp8)
            attv_sb = singles.tile([128, 4], bf16)
            outw_sb = singles.tile([128, 4 * OUT], bf16)
            ident = singles.tile([128, 128], fp8)
            make_identity(nc, ident[:])
            ident32 = singles.tile([32, 32], bf16)
            make_identity(nc, ident32[:])
            one1 = singles.tile([1, 1], bf16)
            nc.vector.memset(one1[:], 1.0)
            convb_sb = attb_sb = outb_sb = ones_sb = None
            if has_conv_b:
                convb_sb = singles.tile([1, NL * F], f32)
                nc.sync.dma_start(convb_sb[:], convb_d[:])
            if has_att_b:
                attb_sb = singles.tile([128, 4], f32)
                nc.sync.dma_start(attb_sb[:], attb_d[:])
            if has_out_b:
                outb_sb = singles.tile([1, OUT], f32)
                nc.sync.dma_start(outb_sb[:], outb_d[:])
                ones_sb = singles.tile([1, 32], f32)
                nc.vector.memset(ones_sb[:], 1.0)

            pstack = singles.tile([GPC, D], bf16)

            # ---------------- Phase A: convs + attention ----------------
            with (
                tc.tile_pool(name="atb", bufs=2) as p_atb,
                tc.tile_pool(name="at8", bufs=2) as p_at8,
                tc.tile_pool(name="h0", bufs=2) as p_h0,
                tc.tile_pool(name="x", bufs=2 * (NL + 1)) as p_x,
                tc.tile_pool(name="hT", bufs=2) as p_hT,
                tc.tile_pool(name="agg", bufs=2) as p_agg,
                tc.tile_pool(name="t", bufs=4) as p_t,
                tc.tile_pool(name="sm", bufs=8) as p_sm,
                tc.tile_pool(name="rc", bufs=2) as p_rc,
                tc.tile_pool(name="ps_main", bufs=4, space="PSUM") as ps_main,
                tc.tile_pool(name="ps_uT", bufs=2, space="PSUM") as ps_uT,
            ):
                xs = {}
                hTs = {}
                recips = {}
                for gp in range(0, GPC, 2):
                    pair = (gp, gp + 1)
                    atb_sbs = {}
                    at8_sbs = {}
                    for gg in pair:
                        atb_t = p_atb.tile([128, 4, D], bf16, tag="atb")
                        at8_t = p_at8.tile([128, 4, D], fp8, tag="at8")
                        atb_sbs[gg] = atb_t
                        at8_sbs[gg] = at8_t
                        h0_t = p_x.tile([128, 4, F], bf16, tag="x")
                        xs[(gg, 0)] = h0_t
                        if gg == 0:
                            # split the first graph's loads per chunk so the
                            # first matmul starts as soon as chunk 0 lands
                            for c in range(4):
                                nc.sync.dma_start(h0_t[:, c, :], h0_d[gg, :, c, :])
                                nc.sync.dma_start(atb_t[:, c, :], atb_d[gg, :, c, :])
                            nc.sync.dma_start(at8_t[:], at8_d[gg])
                        else:
                            nc.sync.dma_start(atb_t[:], atb_d[gg])
                            nc.sync.dma_start(at8_t[:], at8_d[gg])
                            nc.sync.dma_start(h0_t[:], h0_d[gg])
                        if has_conv_b:
                            rc_t = p_rc.tile([1, D], f32)
                            recips[gg] = rc_t
                            nc.sync.dma_start(rc_t[:], recip_d[gg : gg + 1, :])
                    if gp == 0:
                        nc.sync.dma_start(convw_sb[:], convw_d[:])
                        nc.sync.dma_start(attw_sb[:], attw_d[:])
                        nc.sync.dma_start(attv_sb[:], attv_d[:])
                        nc.sync.dma_start(outw_sb[:], outw_d[:])

                    for l in range(NL):
                        agg_pss = {}
                        for gg in pair:
                            agg_ps = ps_main.tile([128, D], f32, tag="main")
                            agg_pss[gg] = agg_ps
                            if l == 0:
                                for c in range(4):
                                    nc.tensor.matmul(
                                        agg_ps[:],
                                        xs[(gg, 0)][:, c, :],
                                        atb_sbs[gg][:, c, :],
                                        start=(c == 0),
                                        stop=(c == 3),
                                    )
                            else:
                                for p in range(2):
                                    nc.tensor.matmul(
                                        agg_ps[:],
                                        xs[(gg, l)][:, 2 * p : 2 * p + 2, :],
                                        at8_sbs[gg][:, 2 * p : 2 * p + 2, :],
                                        start=(p == 0),
                                        stop=(p == 1),
                                        perf_mode=DR,
                                    )
                        for gg in pair:
                            agg_sb = p_agg.tile([128, D], bf16, tag="agg")
                            nc.vector.tensor_copy(agg_sb[:], agg_pss[gg][:])

                            linT_ps = ps_main.tile([128, D], f32, tag="main")
                            if has_conv_b:
                                # linT += conv_b[l] (x) recip_deg  (outer product)
                                nc.tensor.matmul(
                                    linT_ps[:],
                                    convb_sb[0:1, l * F : (l + 1) * F],
                                    recips[gg][:],
                                    start=True,
                                    stop=False,
                                )
                            nc.tensor.matmul(
                                linT_ps[:],
                                convw_sb[:, l, :],
                                agg_sb[:],
                                start=not has_conv_b,
                                stop=True,
                            )
                            if l == 0:
                                hT_all = p_hT.tile([128, NL, D], fp8, tag="hT")
                                hTs[gg] = hT_all
                            nc.scalar.activation(
                                hTs[gg][:, l, :],
                                linT_ps[:],
                                mybir.ActivationFunctionType.Tanh,
                            )
                            # node-major fp8 x_{l+1} via PE transposes of hT
                            # (fp8 transpose mode requires output element step 2)
                            tp_ps = ps_main.tile([128, 2 * D], fp8, tag="main")
                            tpv = tp_ps[:].rearrange(
                                "p (c n two) -> p c n two", c=4, two=2
                            )
                            for c in range(4):
                                nc.tensor.transpose(
                                    tpv[:, c, :, 0],
                                    hTs[gg][:, l, c * F : (c + 1) * F],
                                    ident[:],
                                )
                            x_next = p_x.tile([128, 4, F], fp8, tag="x")
                            xs[(gg, l + 1)] = x_next
                            nc.vector.tensor_copy(x_next[:], tpv[:, :, :, 0])

                    for gg in pair:
                        # ---- attention scores ----
                        t_sbs = []
                        for mh in range(2):  # m half: (m=2mh, 2mh+1)
                            uT_ps = ps_uT.tile([128, 2, D], f32, tag="uT")
                            for mi in range(2):
                                m = 2 * mh + mi
                                for p in range(2):
                                    nc.tensor.matmul(
                                        uT_ps[:, mi, :],
                                        attw_sb[
                                            :, 4 * m + 2 * p : 4 * m + 2 * p + 2, :
                                        ],
                                        hTs[gg][:, 2 * p : 2 * p + 2, :],
                                        start=(p == 0),
                                        stop=(p == 1),
                                        perf_mode=DR,
                                    )
                            t_sb = p_t.tile([128, 2, D], bf16, tag="t")
                            if has_att_b:
                                for mi in range(2):
                                    m = 2 * mh + mi
                                    nc.scalar.activation(
                                        t_sb[:, mi, :],
                                        uT_ps[:, mi, :],
                                        mybir.ActivationFunctionType.Tanh,
                                        bias=attb_sb[:, m : m + 1],
                                        scale=1.0 / ATT_SCALE,
                                    )
                            else:
                                nc.scalar.activation(
                                    t_sb[:],
                                    uT_ps[:],
                                    mybir.ActivationFunctionType.Tanh,
                                    scale=1.0 / ATT_SCALE,
                                )
                            t_sbs.append(t_sb)
                        # 4 concurrent M=1 matmuls on distinct PE column groups
                        s4_ps = ps_main.tile([128, D], f32, tag="main")
                        for m in range(4):
                            nc.tensor.matmul(
                                s4_ps[32 * m : 32 * m + 1, :],
                                attv_sb[:, m : m + 1],
                                t_sbs[m // 2][:, m % 2, :],
                                start=True,
                                stop=True,
                                tile_position=(0, 32 * m),
                            )
                        s_sb = p_sm.tile([1, D], f32, tag="s")
                        nc.vector.tensor_copy(s_sb[:], s4_ps[0:1, :])
                        for rr in (32, 64, 96):
                            nc.vector.tensor_add(
                                s_sb[:], s_sb[:], s4_ps[rr : rr + 1, :]
                            )
                        negmax = p_sm.tile([1, 1], f32, tag="nm")
                        nc.vector.tensor_reduce(
                            negmax[:],
                            s_sb[:],
                            axis=mybir.AxisListType.X,
                            op=mybir.AluOpType.max,
                            negate=True,
                        )
                        attn_u = p_sm.tile([1, D], bf16, tag="attnu")
                        ssum = p_sm.tile([1, 1], f32, tag="ssum")
                        nc.scalar.activation(
                            attn_u[:],
                            s_sb[:],
                            mybir.ActivationFunctionType.Exp,
                            bias=negmax[:],
                            accum_out=ssum[:],
                        )
                        recip_s = p_sm.tile([1, 1], f32, tag="rcs")
                        nc.vector.reciprocal(recip_s[:], ssum[:])
                        # attn column extraction: [1,512] -> [128,4]
                        col_ps = ps_main.tile([128, 4], f32, tag="main")
                        for r in range(4):
                            nc.tensor.matmul(
                                col_ps[:, r : r + 1],
                                attn_u[0:1, r * 128 : (r + 1) * 128],
                                one1[:],
                                start=(r == 0),
                                stop=(r == 3),
                            )
                        attn_col = p_sm.tile([128, 4], bf16, tag="acol")
                        nc.vector.tensor_copy(attn_col[:], col_ps[:])
                        # pooling: 16 column-packed matmuls
                        pooled4_ps = ps_main.tile([128, D], f32, tag="main")
                        for l in range(NL):
                            for r in range(4):
                                nc.tensor.matmul(
                                    pooled4_ps[32 * r : 32 * r + 1, l * F : (l + 1) * F],
                                    attn_col[:, r : r + 1],
                                    xs[(gg, l + 1)][:, r, :],
                                    start=(l == 0),
                                    stop=(l == 3),
                                    tile_position=(0, 32 * r),
                                )
                        po_sum = p_sm.tile([1, D], f32, tag="posum")
                        nc.vector.tensor_copy(po_sum[:], pooled4_ps[0:1, :])
                        for rr in (32, 64, 96):
                            nc.vector.tensor_add(
                                po_sum[:], po_sum[:], pooled4_ps[rr : rr + 1, :]
                            )
                        po_sb = p_sm.tile([1, D], bf16, tag="po")
                        nc.vector.tensor_scalar_mul(po_sb[:], po_sum[:], recip_s[:])
                        nc.sync.dma_start(pstack[gg : gg + 1, :], po_sb[:])

            # ---------------- Phase B: output head --------------------
            with (
                tc.tile_pool(name="fin", bufs=2) as p_fin,
                tc.tile_pool(name="ps_tp", bufs=2, space="PSUM") as ps_tp,
                tc.tile_pool(name="ps_out", bufs=1, space="PSUM") as ps_out,
            ):
                pT = p_fin.tile([128, 4 * GPC], bf16)
                for c in range(4):
                    tp_ps = ps_tp.tile([128, GPC], bf16)
                    nc.tensor.transpose(
                        tp_ps[:], pstack[:, c * 128 : (c + 1) * 128], ident32[:]
                    )
                    nc.scalar.copy(pT[:, c * GPC : (c + 1) * GPC], tp_ps[:])
                out_ps = ps_out.tile([GPC, OUT], f32)
                if has_out_b:
                    nc.tensor.matmul(
                        out_ps[:], ones_sb[:], outb_sb[:], start=True, stop=False
                    )
                for c in range(4):
                    nc.tensor.matmul(
                        out_ps[:],
                        pT[:, c * GPC : (c + 1) * GPC],
                        outw_sb[:, c * OUT : (c + 1) * OUT],
                        start=(c == 0 and not has_out_b),
                        stop=(c == 3),
                    )
                out_sb = p_fin.tile([GPC, OUT], f32)
                nc.scalar.activation(
                    out_sb[:], out_ps[:], mybir.ActivationFunctionType.Relu
                )
                nc.sync.dma_start(out_d[:], out_sb[:])

    nc.compile()
    _NC_CACHE[key] = nc
    return nc


def _prep_inputs(node_feat, edge_src, edge_dst, conv_W, att_W, att_v, out_W):
    src = edge_src.astype(np.int64)
    dst = edge_dst.astype(np.int64)
    ls = src - (dst // N) * N  # src local id within dst's graph
    idx = dst * N + ls
    counts = np.bincount(idx, minlength=B * N * N).astype(np.float32)
    A = counts.reshape(B, N, N)
    iN = np.arange(N)
    A[:, iN, iN] += 1.0
    degs = A.sum(axis=2)  # == deg + 1
    Ahat = A / degs[:, :, None]
    At = np.ascontiguousarray(Ahat.transpose(0, 2, 1))  # [g, src, dst]
    at_host = np.ascontiguousarray(At.reshape(B, 4, 128, N).transpose(0, 2, 1, 3))
    # [B, 128, 4, N]

    h0_host = np.ascontiguousarray(
        node_feat.reshape(B, 4, 128, F).transpose(0, 2, 1, 3)
    )  # [B, 128, 4, F]

    convw2 = np.ascontiguousarray(conv_W.transpose(1, 0, 2))  # [128, NL, F]
    # attw DoubleRow packing: [128, 16, 128] where index m*4 + 2p + j holds
    # att_W rows (2p+j)*128:(2p+j+1)*128, cols m*128:(m+1)*128
    attw_dr = np.ascontiguousarray(
        att_W.reshape(4, 128, 4, 128).transpose(1, 0, 2, 3)  # [128, lc, m, 128]
    )
    attw2 = np.empty((128, 16, 128), dtype=np.float32)
    for m in range(4):
        for p in range(2):
            for j in range(2):
                attw2[:, 4 * m + 2 * p + j, :] = attw_dr[:, 2 * p + j, m, :]
    attv2 = np.ascontiguousarray(att_v.reshape(4, 128).T)
    outw2 = np.ascontiguousarray(
        out_W.reshape(4, 128, OUT).transpose(1, 0, 2)
    ).reshape(128, 4 * OUT)
    return at_host, h0_host, convw2, attw2, attv2, outw2, degs


def kernel(
    node_feat,
    edge_src,
    edge_dst,
    conv_W,
    conv_b,
    att_W,
    att_b,
    att_v,
    out_W,
    out_b,
):
    from concourse.bass_utils import run_bass_kernel_spmd

    at_host, h0_host, convw2, attw2, attv2, outw2, degs = _prep_inputs(
        np.asarray(node_feat, dtype=np.float32),
        np.asarray(edge_src),
        np.asarray(edge_dst),
        np.asarray(conv_W, dtype=np.float32),
        np.asarray(att_W, dtype=np.float32),
        np.asarray(att_v, dtype=np.float32),
        np.asarray(out_W, dtype=np.float32),
    )
    conv_b = np.asarray(conv_b, dtype=np.float32)
    att_b = np.asarray(att_b, dtype=np.float32)
    out_b = np.asarray(out_b, dtype=np.float32)
    has_conv_b = bool(np.any(conv_b))
    has_att_b = bool(np.any(att_b))
    has_out_b = bool(np.any(out_b))

    nc = _build_nc(has_conv_b, has_att_b, has_out_b)

    atb_h = at_host.astype(BF16)
    at8_h = at_host.astype(FP8)
    convw_b = convw2.astype(BF16)
    attw_b = (attw2 * ATT_SCALE).astype(FP8)
    attv_b = attv2.astype(BF16)
    outw_b = outw2.astype(BF16)

    in_maps = []
    for c in range(NCORES):
        sl = slice(c * GPC, (c + 1) * GPC)
        m = {
            "atb": atb_h[sl],
            "at8": at8_h[sl],
            "h0": h0_host[sl].astype(BF16),
            "convw": convw_b,
            "attw": attw_b,
            "attv": attv_b,
            "outw": outw_b,
        }
        if has_conv_b:
            m["convb"] = conv_b.reshape(1, NL * F)
            m["recipdeg"] = (1.0 / degs[sl]).astype(np.float32)
        if has_att_b:
            m["attb"] = np.ascontiguousarray(att_b.reshape(4, 128).T)
        if has_out_b:
            m["outb"] = out_b.reshape(1, OUT)
        in_maps.append(m)

    res = run_bass_kernel_spmd(nc, in_maps, core_ids=list(range(NCORES)))
    out = np.concatenate([r["out"] for r in res.results], axis=0)
    return np.ascontiguousarray(out.astype(np.float32))


# revision 17
# speedup vs baseline: 1.1179x; 1.1179x over previous
"""Trainium2 Bass kernel for nn_AttPool (4-layer GNN + additive-attention pooling).

Strategy (data-parallel over graphs, 32 graphs per NeuronCore):
  * Host re-lays-out the edge list as per-graph dense normalized adjacency
    Ahat^T = ((A + I) / deg)^T, shipped in bf16 (layer 0) and fp8-e4m3
    (layers 1-3).
  * Per graph g, per layer l:
      - aggT = sum_c h_c^T @ AhatT_c.  Layer 0 runs in bf16 (4 matmuls,
        h0 stays bf16 — quantizing the raw features dominates fp8 error);
        layers 1-3 run as 2 fp8 DoubleRow matmuls (K=256 per pass, both
        operands fp8) at ~2x column throughput.
      - linT = convW_l^T @ aggT (feat-major, single matmul, single weight
        load), then one tanh ACT emits hT_l directly in fp8.
      - node-major x_l (fp8) comes from 4 PE transposes of hT_l (post-tanh,
        so no second activation), feeding the next layer's spmm and pooling.
  * Attention: uT_m accumulated via fp8 DoubleRow matmuls (attW pre-packed
    x64 in fp8, descaled inside the tanh ACT's scale); t in two fused
    [128,1024] ACTs; scores via 4 column-packed M=1 matmuls + DVE row adds;
    softmax per graph; pooling via column-packed matmuls + DVE row adds.
  * Tiny epilogue: transpose pooled rows, output head matmul, ReLU.
  Measured end-to-end rel-err vs fp32 reference: ~1e-2 (gate 2e-2).
"""

import numpy as np
import ml_dtypes

B, N, F = 256, 512, 128
NL = 4
D = 512
OUT = 128
NCORES = 8
GPC = B // NCORES  # graphs per core

BF16 = ml_dtypes.bfloat16
FP8 = ml_dtypes.float8_e4m3
ATT_SCALE = 64.0

_NC_CACHE = {}


def _build_nc(has_conv_b, has_att_b, has_out_b):
    key = (has_conv_b, has_att_b, has_out_b)
    if key in _NC_CACHE:
        return _NC_CACHE[key]

    import concourse.bacc as bacc
    import concourse.tile as tile
    import concourse.mybir as mybir
    from concourse.masks import make_identity

    f32 = mybir.dt.float32
    bf16 = mybir.dt.bfloat16
    fp8 = mybir.dt.float8e4
    DR = mybir.MatmulPerfMode.DoubleRow

    nc = bacc.Bacc(None, target_bir_lowering=False)

    atb_d = nc.dram_tensor("atb", [GPC, 128, 4, D], bf16, kind="ExternalInput")
    at8_d = nc.dram_tensor("at8", [GPC, 128, 4, D], fp8, kind="ExternalInput")
    h0_d = nc.dram_tensor("h0", [GPC, 128, 4, F], bf16, kind="ExternalInput")
    convw_d = nc.dram_tensor("convw", [128, NL, F], bf16, kind="ExternalInput")
    attw_d = nc.dram_tensor("attw", [128, 16, F], fp8, kind="ExternalInput")
    attv_d = nc.dram_tensor("attv", [128, 4], bf16, kind="ExternalInput")
    outw_d = nc.dram_tensor("outw", [128, 4 * OUT], bf16, kind="ExternalInput")
    out_d = nc.dram_tensor("out", [GPC, OUT], f32, kind="ExternalOutput")
    convb_d = recip_d = attb_d = outb_d = None
    if has_conv_b:
        convb_d = nc.dram_tensor("convb", [1, NL * F], f32, kind="ExternalInput")
        recip_d = nc.dram_tensor("recipdeg", [GPC, D], f32, kind="ExternalInput")
    if has_att_b:
        attb_d = nc.dram_tensor("attb", [128, 4], f32, kind="ExternalInput")
    if has_out_b:
        outb_d = nc.dram_tensor("outb", [1, OUT], f32, kind="ExternalInput")

    with tile.TileContext(nc) as tc:
        with (
            tc.tile_pool(name="singles", bufs=1) as singles,
        ):
            convw_sb = singles.tile([128, NL, F], bf16)
            attw_sb = singles.tile([128, 16, F], fp8)
            attv_sb = singles.tile([128, 4], bf16)
            outw_sb = singles.tile([128, 4 * OUT], bf16)
            ident = singles.tile([128, 128], fp8)
            make_identity(nc, ident[:])
            ident32 = singles.tile([32, 32], bf16)
            make_identity(nc, ident32[:])
            one1 = singles.tile([1, 1], bf16)
            nc.vector.memset(one1[:], 1.0)
            ones128 = singles.tile([128, 1], bf16)
            nc.vector.memset(ones128[:], 1.0)
            convb_sb = attb_sb = outb_sb = ones_sb = None
            if has_conv_b:
                convb_sb = singles.tile([1, NL * F], f32)
                nc.sync.dma_start(convb_sb[:], convb_d[:])
            if has_att_b:
                attb_sb = singles.tile([128, 4], f32)
                nc.sync.dma_start(attb_sb[:], attb_d[:])
            if has_out_b:
                outb_sb = singles.tile([1, OUT], f32)
                nc.sync.dma_start(outb_sb[:], outb_d[:])
                ones_sb = singles.tile([1, 32], f32)
                nc.vector.memset(ones_sb[:], 1.0)

            pstack = singles.tile([GPC, D], bf16)

            # ---------------- Phase A: convs + attention ----------------
            with (
                tc.tile_pool(name="atb", bufs=4) as p_atb,
                tc.tile_pool(name="at8", bufs=4) as p_at8,
                tc.tile_pool(name="h0", bufs=4) as p_h0,
                tc.tile_pool(name="x", bufs=2 * NL) as p_x,
                tc.tile_pool(name="hT", bufs=2) as p_hT,
                tc.tile_pool(name="agg", bufs=2) as p_agg,
                tc.tile_pool(name="t", bufs=4) as p_t,
                tc.tile_pool(name="sm", bufs=8) as p_sm,
                tc.tile_pool(name="rc", bufs=4) as p_rc,
                tc.tile_pool(name="ps_main", bufs=4, space="PSUM") as ps_main,
                tc.tile_pool(name="ps_uT", bufs=2, space="PSUM") as ps_uT,
            ):
                xs = {}
                hTs = {}
                recips = {}
                atb_sbs = {}
                at8_sbs = {}

                def issue_dma(gg, split_first):
                    atb_t = p_atb.tile([128, 4, D], bf16, tag="atb")
                    at8_t = p_at8.tile([128, 4, D], fp8, tag="at8")
                    atb_sbs[gg] = atb_t
                    at8_sbs[gg] = at8_t
                    h0_t = p_h0.tile([128, 4, F], bf16, tag="h0")
                    xs[(gg, 0)] = h0_t
                    if split_first:
                        # split the first graph's loads per chunk so the
                        # first matmul starts as soon as chunk 0 lands
                        for c in range(4):
                            nc.sync.dma_start(h0_t[:, c, :], h0_d[gg, :, c, :])
                            nc.sync.dma_start(atb_t[:, c, :], atb_d[gg, :, c, :])
                        nc.sync.dma_start(at8_t[:], at8_d[gg])
                    else:
                        nc.sync.dma_start(atb_t[:], atb_d[gg])
                        nc.sync.dma_start(at8_t[:], at8_d[gg])
                        nc.sync.dma_start(h0_t[:], h0_d[gg])
                    if has_conv_b:
                        rc_t = p_rc.tile([1, D], f32)
                        recips[gg] = rc_t
                        nc.sync.dma_start(rc_t[:], recip_d[gg : gg + 1, :])

                for gp in range(0, GPC, 2):
                    pair = (gp, gp + 1)
                    if gp == 0:
                        issue_dma(0, True)
                        issue_dma(1, False)
                        nc.sync.dma_start(convw_sb[:], convw_d[:])
                        nc.sync.dma_start(attw_sb[:], attw_d[:])
                        nc.sync.dma_start(attv_sb[:], attv_d[:])
                        nc.sync.dma_start(outw_sb[:], outw_d[:])
                    # prefetch the NEXT pair's tensors while this pair computes
                    if gp + 2 < GPC:
                        issue_dma(gp + 2, False)
                        issue_dma(gp + 3, False)

                    for l in range(NL):
                        agg_pss = {}
                        for gg in pair:
                            agg_ps = ps_main.tile([128, D], f32, tag="main")
                            agg_pss[gg] = agg_ps
                            if l == 0:
                                for c in range(4):
                                    nc.tensor.matmul(
                                        agg_ps[:],
                                        xs[(gg, 0)][:, c, :],
                                        atb_sbs[gg][:, c, :],
                                        start=(c == 0),
                                        stop=(c == 3),
                                    )
                            else:
                                for p in range(2):
                                    nc.tensor.matmul(
                                        agg_ps[:],
                                        xs[(gg, l)][:, 2 * p : 2 * p + 2, :],
                                        at8_sbs[gg][:, 2 * p : 2 * p + 2, :],
                                        start=(p == 0),
                                        stop=(p == 1),
                                        perf_mode=DR,
                                    )
                        for gg in pair:
                            agg_sb = p_agg.tile([128, D], bf16, tag="agg")
                            nc.vector.tensor_copy(agg_sb[:], agg_pss[gg][:])

                            linT_ps = ps_main.tile([128, D], f32, tag="main")
                            if has_conv_b:
                                # linT += conv_b[l] (x) recip_deg  (outer product)
                                nc.tensor.matmul(
                                    linT_ps[:],
                                    convb_sb[0:1, l * F : (l + 1) * F],
                                    recips[gg][:],
                                    start=True,
                                    stop=False,
                                )
                            nc.tensor.matmul(
                                linT_ps[:],
                                convw_sb[:, l, :],
                                agg_sb[:],
                                start=not has_conv_b,
                                stop=True,
                            )
                            if l == 0:
                                hT_all = p_hT.tile([128, NL, D], fp8, tag="hT")
                                hTs[gg] = hT_all
                            nc.scalar.activation(
                                hTs[gg][:, l, :],
                                linT_ps[:],
                                mybir.ActivationFunctionType.Tanh,
                            )
                            # node-major fp8 x_{l+1} via PE transposes of hT
                            # (fp8 transpose mode requires output element step 2)
                            tp_ps = ps_main.tile([128, 2 * D], fp8, tag="main")
                            tpv = tp_ps[:].rearrange(
                                "p (c n two) -> p c n two", c=4, two=2
                            )
                            for c in range(4):
                                nc.tensor.transpose(
                                    tpv[:, c, :, 0],
                                    hTs[gg][:, l, c * F : (c + 1) * F],
                                    ident[:],
                                )
                            x_next = p_x.tile([128, 4, F], fp8, tag="x")
                            xs[(gg, l + 1)] = x_next
                            nc.vector.tensor_copy(x_next[:], tpv[:, :, :, 0])

                    for gg in pair:
                        # ---- attention scores ----
                        t_sbs = []
                        for mh in range(2):  # m half: (m=2mh, 2mh+1)
                            uT_ps = ps_uT.tile([128, 2, D], f32, tag="uT")
                            for mi in range(2):
                                m = 2 * mh + mi
                                for p in range(2):
                                    nc.tensor.matmul(
                                        uT_ps[:, mi, :],
                                        attw_sb[
                                            :, 4 * m + 2 * p : 4 * m + 2 * p + 2, :
                                        ],
                                        hTs[gg][:, 2 * p : 2 * p + 2, :],
                                        start=(p == 0),
                                        stop=(p == 1),
                                        perf_mode=DR,
                                    )
                            t_sb = p_t.tile([128, 2, D], bf16, tag="t")
                            if has_att_b:
                                for mi in range(2):
                                    m = 2 * mh + mi
                                    nc.scalar.activation(
                                        t_sb[:, mi, :],
                                        uT_ps[:, mi, :],
                                        mybir.ActivationFunctionType.Tanh,
                                        bias=attb_sb[:, m : m + 1],
                                        scale=1.0 / ATT_SCALE,
                                    )
                            else:
                                nc.scalar.activation(
                                    t_sb[:],
                                    uT_ps[:],
                                    mybir.ActivationFunctionType.Tanh,
                                    scale=1.0 / ATT_SCALE,
                                )
                            t_sbs.append(t_sb)
                        # 4 concurrent M=1 matmuls on distinct PE column groups
                        s4_ps = ps_main.tile([128, D], f32, tag="main")
                        nc.vector.memset(s4_ps[:], 0.0)
                        for m in range(4):
                            nc.tensor.matmul(
                                s4_ps[32 * m : 32 * m + 1, :],
                                attv_sb[:, m : m + 1],
                                t_sbs[m // 2][:, m % 2, :],
                                start=True,
                                stop=True,
                                tile_position=(0, 32 * m),
                            )
                        s4_sb = p_t.tile([128, D], bf16, tag="s4")
                        nc.vector.tensor_copy(s4_sb[:], s4_ps[:])
                        s_ps = ps_main.tile([1, D], f32, tag="main")
                        nc.tensor.matmul(
                            s_ps[:], ones128[:], s4_sb[:], start=True, stop=True
                        )
                        negmax = p_sm.tile([1, 1], f32, tag="nm")
                        nc.vector.tensor_reduce(
                            negmax[:],
                            s_ps[:],
                            axis=mybir.AxisListType.X,
                            op=mybir.AluOpType.max,
                            negate=True,
                        )
                        attn_u = p_sm.tile([1, D], bf16, tag="attnu")
                        ssum = p_sm.tile([1, 1], f32, tag="ssum")
                        nc.scalar.activation(
                            attn_u[:],
                            s_ps[:],
                            mybir.ActivationFunctionType.Exp,
                            bias=negmax[:],
                            accum_out=ssum[:],
                        )
                        recip_s = p_sm.tile([1, 1], f32, tag="rcs")
                        nc.vector.reciprocal(recip_s[:], ssum[:])
                        # attn column extraction: [1,512] -> [128,4]
                        col_ps = ps_main.tile([128, 4], f32, tag="main")
                        for r in range(4):
                            nc.tensor.matmul(
                                col_ps[:, r : r + 1],
                                attn_u[0:1, r * 128 : (r + 1) * 128],
                                one1[:],
                                start=(r == 0),
                                stop=(r == 3),
                            )
                        attn_col = p_sm.tile([128, 4], bf16, tag="acol")
                        nc.vector.tensor_copy(attn_col[:], col_ps[:])
                        # pooling: 16 column-packed matmuls
                        pooled4_ps = ps_main.tile([128, D], f32, tag="main")
                        nc.vector.memset(pooled4_ps[:], 0.0)
                        for l in range(NL):
                            for r in range(4):
                                nc.tensor.matmul(
                                    pooled4_ps[32 * r : 32 * r + 1, l * F : (l + 1) * F],
                                    attn_col[:, r : r + 1],
                                    xs[(gg, l + 1)][:, r, :],
                                    start=(l == 0),
                                    stop=(l == 3),
                                    tile_position=(0, 32 * r),
                                )
                        pooled4_sb = p_t.tile([128, D], bf16, tag="s4")
                        nc.vector.tensor_copy(pooled4_sb[:], pooled4_ps[:])
                        pooled_ps = ps_main.tile([1, D], f32, tag="main")
                        nc.tensor.matmul(
                            pooled_ps[:], ones128[:], pooled4_sb[:], start=True, stop=True
                        )
                        po_sb = p_sm.tile([1, D], bf16, tag="po")
                        nc.vector.tensor_scalar_mul(po_sb[:], pooled_ps[:], recip_s[:])
                        nc.sync.dma_start(pstack[gg : gg + 1, :], po_sb[:])

            # ---------------- Phase B: output head --------------------
            with (
                tc.tile_pool(name="fin", bufs=2) as p_fin,
                tc.tile_pool(name="ps_tp", bufs=2, space="PSUM") as ps_tp,
                tc.tile_pool(name="ps_out", bufs=1, space="PSUM") as ps_out,
            ):
                pT = p_fin.tile([128, 4 * GPC], bf16)
                for c in range(4):
                    tp_ps = ps_tp.tile([128, GPC], bf16)
                    nc.tensor.transpose(
                        tp_ps[:], pstack[:, c * 128 : (c + 1) * 128], ident32[:]
                    )
                    nc.scalar.copy(pT[:, c * GPC : (c + 1) * GPC], tp_ps[:])
                out_ps = ps_out.tile([GPC, OUT], f32)
                if has_out_b:
                    nc.tensor.matmul(
                        out_ps[:], ones_sb[:], outb_sb[:], start=True, stop=False
                    )
                for c in range(4):
                    nc.tensor.matmul(
                        out_ps[:],
                        pT[:, c * GPC : (c + 1) * GPC],
                        outw_sb[:, c * OUT : (c + 1) * OUT],
                        start=(c == 0 and not has_out_b),
                        stop=(c == 3),
                    )
                out_sb = p_fin.tile([GPC, OUT], f32)
                nc.scalar.activation(
                    out_sb[:], out_ps[:], mybir.ActivationFunctionType.Relu
                )
                nc.sync.dma_start(out_d[:], out_sb[:])

    nc.compile()
    _NC_CACHE[key] = nc
    return nc


def _prep_inputs(node_feat, edge_src, edge_dst, conv_W, att_W, att_v, out_W):
    src = edge_src.astype(np.int64)
    dst = edge_dst.astype(np.int64)
    ls = src - (dst // N) * N  # src local id within dst's graph
    idx = dst * N + ls
    counts = np.bincount(idx, minlength=B * N * N).astype(np.float32)
    A = counts.reshape(B, N, N)
    iN = np.arange(N)
    A[:, iN, iN] += 1.0
    degs = A.sum(axis=2)  # == deg + 1
    Ahat = A / degs[:, :, None]
    At = np.ascontiguousarray(Ahat.transpose(0, 2, 1))  # [g, src, dst]
    at_host = np.ascontiguousarray(At.reshape(B, 4, 128, N).transpose(0, 2, 1, 3))
    # [B, 128, 4, N]

    h0_host = np.ascontiguousarray(
        node_feat.reshape(B, 4, 128, F).transpose(0, 2, 1, 3)
    )  # [B, 128, 4, F]

    convw2 = np.ascontiguousarray(conv_W.transpose(1, 0, 2))  # [128, NL, F]
    # attw DoubleRow packing: [128, 16, 128] where index m*4 + 2p + j holds
    # att_W rows (2p+j)*128:(2p+j+1)*128, cols m*128:(m+1)*128
    attw_dr = np.ascontiguousarray(
        att_W.reshape(4, 128, 4, 128).transpose(1, 0, 2, 3)  # [128, lc, m, 128]
    )
    attw2 = np.empty((128, 16, 128), dtype=np.float32)
    for m in range(4):
        for p in range(2):
            for j in range(2):
                attw2[:, 4 * m + 2 * p + j, :] = attw_dr[:, 2 * p + j, m, :]
    attv2 = np.ascontiguousarray(att_v.reshape(4, 128).T)
    outw2 = np.ascontiguousarray(
        out_W.reshape(4, 128, OUT).transpose(1, 0, 2)
    ).reshape(128, 4 * OUT)
    return at_host, h0_host, convw2, attw2, attv2, outw2, degs


def kernel(
    node_feat,
    edge_src,
    edge_dst,
    conv_W,
    conv_b,
    att_W,
    att_b,
    att_v,
    out_W,
    out_b,
):
    from concourse.bass_utils import run_bass_kernel_spmd

    at_host, h0_host, convw2, attw2, attv2, outw2, degs = _prep_inputs(
        np.asarray(node_feat, dtype=np.float32),
        np.asarray(edge_src),
        np.asarray(edge_dst),
        np.asarray(conv_W, dtype=np.float32),
        np.asarray(att_W, dtype=np.float32),
        np.asarray(att_v, dtype=np.float32),
        np.asarray(out_W, dtype=np.float32),
    )
    conv_b = np.asarray(conv_b, dtype=np.float32)
    att_b = np.asarray(att_b, dtype=np.float32)
    out_b = np.asarray(out_b, dtype=np.float32)
    has_conv_b = bool(np.any(conv_b))
    has_att_b = bool(np.any(att_b))
    has_out_b = bool(np.any(out_b))

    nc = _build_nc(has_conv_b, has_att_b, has_out_b)

    atb_h = at_host.astype(BF16)
    at8_h = at_host.astype(FP8)
    convw_b = convw2.astype(BF16)
    attw_b = (attw2 * ATT_SCALE).astype(FP8)
    attv_b = attv2.astype(BF16)
    outw_b = outw2.astype(BF16)

    in_maps = []
    for c in range(NCORES):
        sl = slice(c * GPC, (c + 1) * GPC)
        m = {
            "atb": atb_h[sl],
            "at8": at8_h[sl],
            "h0": h0_host[sl].astype(BF16),
            "convw": convw_b,
            "attw": attw_b,
            "attv": attv_b,
            "outw": outw_b,
        }
        if has_conv_b:
            m["convb"] = conv_b.reshape(1, NL * F)
            m["recipdeg"] = (1.0 / degs[sl]).astype(np.float32)
        if has_att_b:
            m["attb"] = np.ascontiguousarray(att_b.reshape(4, 128).T)
        if has_out_b:
            m["outb"] = out_b.reshape(1, OUT)
        in_maps.append(m)

    res = run_bass_kernel_spmd(nc, in_maps, core_ids=list(range(NCORES)))
    out = np.concatenate([r["out"] for r in res.results], axis=0)
    return np.ascontiguousarray(out.astype(np.float32))
